# revision 13
# baseline (speedup 1.0000x reference)
"""DistMult bilinear scoring kernel for Trainium2 (8 NeuronCores).

scores[e] = left_emb[e] @ W[r_id[e]] @ right_emb[e]

Strategy:
  Host: stable-sort edges by relation (data-parallel shard over 8 cores),
        pad each relation bucket to 512-edge chunks, and pre-layout L/R into
        the transposed block format the PE wants ([dim on partitions]).
        The relation of every 512-edge chunk is baked into the (per-call
        compiled) kernel as a static weight-slice schedule.
  Device (identical program on all 8 cores):
        per 8192-edge macro tile ([128, 2048] f32):
          - DMA in Lt, Rt tiles
          - 16 fp32 matmuls (K=32, M=32, N=512): V.T = W[r].T-apply over Lt
            at diagonal tile positions (4 concurrent 32x32 PE tiles)
          - DVE: Z = V.T (*) Rt  elementwise
          - PE: block-ones matmul reduces each 32-partition block -> scores
          - ACT: copy scores PSUM->SBUF; DMA out contiguous score groups
  Host: inverse-permute scores back to the original edge order.
"""

import math
import os
import sys

import numpy as np

for _p in ("/opt/trn_rl_repo", "/root/.axon_site/_ro/trn_rl_repo"):
    if os.path.isdir(_p) and _p not in sys.path:
        sys.path.insert(0, _p)
        break

import concourse.bass as bass
import concourse.mybir as mybir
import concourse.tile as tile
from concourse import bacc, bass_utils

F32 = mybir.dt.float32

DIM = 32
NUM_REL = 8
N_CORES = 8
CHUNK = 512                      # edges per matmul (fp32 moving-operand max)
TILE_FREE = 2048                 # free dim of a macro tile
TILE_E = 4 * TILE_FREE           # 8192 edges per macro tile
CHUNKS_PER_TILE = TILE_E // CHUNK  # 16
GROUP_TILES = 8                  # macro tiles per score-output group
GROUP_E = GROUP_TILES * TILE_E   # 65536

_module_cache = {}
LAST_RESULTS = None  # BassKernelResults of the most recent run (for test.py)
_hooks_installed = False


def _ensure_profiling_hooks():
    """Make trace=True work in this container: install the NTFF profile hook
    (ctypes into libaxon_pjrt.so, same ABI trn_boot uses) and no-op the S3
    artifact upload."""
    global _hooks_installed
    if _hooks_installed:
        return
    _hooks_installed = True
    bass_utils.upload_artifacts = lambda tmpdir: str(tmpdir)
    try:
        import antenv.axon_hooks  # noqa: F401

        return
    except ImportError:
        pass
    import contextlib
    import ctypes
    import types

    hook = None
    so_path = "/opt/axon/libaxon_pjrt.so"
    if os.path.exists(so_path):
        lib = ctypes.CDLL(so_path)
        if hasattr(lib, "axon_start_nrt_profile"):
            lib.axon_start_nrt_profile.argtypes = [
                ctypes.POINTER(ctypes.c_int64),
                ctypes.c_size_t,
            ]
            lib.axon_start_nrt_profile.restype = ctypes.c_int64
            lib.axon_stop_nrt_profile.argtypes = [ctypes.c_char_p]
            lib.axon_stop_nrt_profile.restype = ctypes.c_int64

            @contextlib.contextmanager
            def _hook(output_dir, device_ids):
                import jax

                jax.devices()
                if device_ids:
                    ids = (ctypes.c_int64 * len(device_ids))(*device_ids)
                    rc = lib.axon_start_nrt_profile(ids, len(device_ids))
                else:
                    rc = lib.axon_start_nrt_profile(None, 0)
                if rc != 0:
                    raise RuntimeError(f"axon_start_nrt_profile rc={rc}")
                try:
                    yield
                finally:
                    n = lib.axon_stop_nrt_profile(str(output_dir).encode())
                    print(f"profile: {n} ntff file(s) in {output_dir}", file=sys.stderr)

            hook = _hook

    mod = types.ModuleType("antenv.axon_hooks")
    mod._hook = hook
    mod.get_axon_ntff_profile_hook = lambda: mod._hook

    def _set(h):
        mod._hook = h

    mod.set_axon_ntff_profile_hook = _set
    import antenv

    sys.modules["antenv.axon_hooks"] = mod
    antenv.axon_hooks = mod


def _build_module(n_tiles: int, rel_sched: tuple):
    """Build the single-core Bass program (same program runs on all 8 cores)."""
    nc = bacc.Bacc(None, target_bir_lowering=False)
    n_groups = math.ceil(n_tiles / GROUP_TILES)

    lt_d = nc.dram_tensor("lt", (n_tiles, 128, TILE_FREE), F32, kind="ExternalInput")
    rt_d = nc.dram_tensor("rt", (n_tiles, 128, TILE_FREE), F32, kind="ExternalInput")
    w_d = nc.dram_tensor("wrep", (128, NUM_REL * DIM), F32, kind="ExternalInput")
    o_d = nc.dram_tensor("onesb", (128, 32), F32, kind="ExternalInput")
    s_d = nc.dram_tensor(
        "scores", (n_groups, 128, GROUP_TILES * CHUNK), F32, kind="ExternalOutput"
    )

    with tile.TileContext(nc) as tc:
        with (
            tc.tile_pool(name="const", bufs=1) as cpool,
            tc.tile_pool(name="io", bufs=3) as iop,
            tc.tile_pool(name="zp", bufs=10) as zp,
            tc.tile_pool(name="sp", bufs=2) as sp,
            tc.tile_pool(name="vps", bufs=4, space="PSUM") as vpool,
            tc.tile_pool(name="sps", bufs=2, space="PSUM") as spool,
        ):
            wrep = cpool.tile([128, NUM_REL * DIM], F32, name="wrep_sb")
            nc.sync.dma_start(wrep[:], w_d[:])
            onesb = cpool.tile([128, 32], F32, name="onesb_sb")
            nc.sync.dma_start(onesb[:], o_d[:])

            state = {"s_sbuf": None}

            def flush(z_list, t_prev):
                # reduce + copy-out for macro tile t_prev (pipelined one tile
                # behind so the PE never stalls waiting on DVE)
                m = t_prev % GROUP_TILES
                if m == 0:
                    state["s_sbuf"] = sp.tile(
                        [128, GROUP_TILES * CHUNK], F32, tag="s", name="s_sb"
                    )
                s_sbuf = state["s_sbuf"]
                s_ps = spool.tile([128, CHUNK], F32, tag="sps", name="s_ps")
                for h in range(4):
                    # rows 32h..32h+4 hold scores; rows +4..+32 are zeros
                    # (zero-padded ones matrix) so everything is initialized
                    nc.tensor.matmul(
                        s_ps[32 * h : 32 * h + 32, :],
                        onesb[:, :],
                        z_list[h][:, :],
                        tile_position=(0, 32 * h),
                    )
                for h in range(4):
                    nc.scalar.copy(
                        s_sbuf[32 * h : 32 * h + 32, CHUNK * m : CHUNK * (m + 1)],
                        s_ps[32 * h : 32 * h + 32, :],
                    )
                if m == GROUP_TILES - 1 or t_prev == n_tiles - 1:
                    g = t_prev // GROUP_TILES
                    cols = CHUNK * (m + 1)
                    nc.sync.dma_start(s_d[g, :, 0:cols], s_sbuf[:, 0:cols])

            pending = None
            for t in range(n_tiles):
                lt = iop.tile([128, TILE_FREE], F32, tag="lt", name="lt_sb")
                nc.sync.dma_start(lt[:], lt_d[t])
                rt = iop.tile([128, TILE_FREE], F32, tag="rt", name="rt_sb")
                nc.sync.dma_start(rt[:], rt_d[t])

                vps = []
                for h in range(4):
                    vp = vpool.tile([128, CHUNK], F32, tag="v", name="v_ps")
                    for pb in range(4):
                        r = rel_sched[t * CHUNKS_PER_TILE + 4 * pb + h]
                        nc.tensor.matmul(
                            vp[32 * pb : 32 * pb + 32, :],
                            wrep[32 * pb : 32 * pb + 32, DIM * r : DIM * (r + 1)],
                            lt[32 * pb : 32 * pb + 32, CHUNK * h : CHUNK * (h + 1)],
                            tile_position=(32 * pb, 32 * pb),
                        )
                    vps.append(vp)

                if pending is not None:
                    flush(*pending)

                z_list = []
                for h in range(4):
                    z = zp.tile([128, CHUNK], F32, tag="z", name="z_sb")
                    nc.vector.tensor_tensor(
                        z[:],
                        vps[h][:],
                        rt[:, CHUNK * h : CHUNK * (h + 1)],
                        op=mybir.AluOpType.mult,
                    )
                    z_list.append(z)
                pending = (z_list, t)

            flush(*pending)
    nc.finalize()
    return nc


def _prep_inputs(left, right, rid):
    """Sort/pad/shard/relayout on the host. Returns device arrays + recovery info."""
    E = left.shape[0]
    perm = np.argsort(rid, kind="stable")
    counts = np.bincount(rid, minlength=NUM_REL).astype(np.int64)
    starts = np.zeros(NUM_REL + 1, dtype=np.int64)
    np.cumsum(counts, out=starts[1:])

    # per-core segment length per relation, multiple of CHUNK
    seg = [
        int(math.ceil(c / (N_CORES * CHUNK))) * CHUNK if c > 0 else 0 for c in counts
    ]
    per_core_real = int(sum(seg))
    n_tiles = max(1, math.ceil(per_core_real / TILE_E))
    T = n_tiles * TILE_E

    # static relation schedule of each 512-chunk (identical on every core)
    rel_sched = []
    for r in range(NUM_REL):
        rel_sched += [r] * (seg[r] // CHUNK)
    rel_sched += [0] * ((T - per_core_real) // CHUNK)
    assert len(rel_sched) == T // CHUNK

    # gather index (into sorted order) for each device slot; -1 = padding
    gidx = np.full((N_CORES, T), -1, dtype=np.int64)
    off = 0
    for r in range(NUM_REL):
        s = seg[r]
        if s == 0:
            continue
        ar = np.arange(s, dtype=np.int64)
        for c in range(N_CORES):
            src = c * s + ar
            gidx[c, off : off + s] = np.where(src < counts[r], starts[r] + src, -1)
        off += s

    L_s = left[perm]
    R_s = right[perm]

    Lt = np.zeros((N_CORES, n_tiles, 128, TILE_FREE), np.float32)
    Rt = np.zeros((N_CORES, n_tiles, 128, TILE_FREE), np.float32)
    for c in range(N_CORES):
        gi = gidx[c]
        msk = gi >= 0
        Lc = np.zeros((T, DIM), np.float32)
        Rc = np.zeros((T, DIM), np.float32)
        Lc[msk] = L_s[gi[msk]]
        Rc[msk] = R_s[gi[msk]]
        # device layout: [tile, 32*pb+k, 512*h+n] = src[tile*8192 + 2048*pb + 512*h + n, k]
        Lt[c] = (
            Lc.reshape(n_tiles, 4, 4, CHUNK, DIM)
            .transpose(0, 1, 4, 2, 3)
            .reshape(n_tiles, 128, TILE_FREE)
        )
        Rt[c] = (
            Rc.reshape(n_tiles, 4, 4, CHUNK, DIM)
            .transpose(0, 1, 4, 2, 3)
            .reshape(n_tiles, 128, TILE_FREE)
        )
    return perm, gidx, n_tiles, tuple(rel_sched), Lt, Rt


def _recover_scores(results, perm, gidx, n_tiles, E):
    T = n_tiles * TILE_E
    n_groups = math.ceil(n_tiles / GROUP_TILES)
    scores_sorted = np.zeros(E, np.float32)
    for c in range(N_CORES):
        sc = np.asarray(results[c]["scores"], dtype=np.float32)
        # [g, 32h+j (j<4), 512m+n] -> sorted pos g*65536 + 8192m + 2048j + 512h + n
        sc2 = (
            sc.reshape(n_groups, 4, 32, GROUP_TILES, CHUNK)[:, :, 0:4]
            .transpose(0, 3, 2, 1, 4)
            .reshape(n_groups * GROUP_E)[:T]
        )
        gi = gidx[c]
        msk = gi >= 0
        scores_sorted[gi[msk]] = sc2[msk]
    scores = np.empty(E, np.float32)
    scores[perm] = scores_sorted
    return scores


def kernel(left_emb, right_emb, r_id, W):
    global LAST_RESULTS
    left = np.ascontiguousarray(np.asarray(left_emb, dtype=np.float32))
    right = np.ascontiguousarray(np.asarray(right_emb, dtype=np.float32))
    rid = np.asarray(r_id).astype(np.int64)
    Wn = np.asarray(W, dtype=np.float32)
    E = left.shape[0]

    perm, gidx, n_tiles, rel_sched, Lt, Rt = _prep_inputs(left, right, rid)

    # W replicated across the 4 partition blocks: wrep[32pb+k, 32r+d] = W[r,k,d]
    wrep = np.tile(Wn.transpose(1, 0, 2).reshape(DIM, NUM_REL * DIM), (4, 1)).astype(
        np.float32
    )
    onesb = np.zeros((128, 32), np.float32)
    for j in range(4):
        onesb[32 * j : 32 * j + 32, j] = 1.0

    key = (n_tiles, rel_sched)
    if key not in _module_cache:
        _module_cache.clear()
        _module_cache[key] = _build_module(n_tiles, rel_sched)
    nc = _module_cache[key]

    in_maps = [
        {"lt": Lt[c], "rt": Rt[c], "wrep": wrep, "onesb": onesb}
        for c in range(N_CORES)
    ]
    trace = bool(int(os.environ.get("KERNEL_TRACE", "0")))
    kwargs = {}
    if trace:
        _ensure_profiling_hooks()
        tdir = os.environ.get("KERNEL_TRACE_DIR")
        if tdir:
            os.makedirs(tdir, exist_ok=True)
            kwargs["tmpdir"] = tdir
    res = bass_utils.run_bass_kernel_spmd(
        nc, in_maps, core_ids=list(range(N_CORES)), trace=trace, **kwargs
    )
    LAST_RESULTS = res
    return _recover_scores(res.results, perm, gidx, n_tiles, E)
